# revision 15
# baseline (speedup 1.0000x reference)
"""Trainium2 Bass kernel for nn_Attention_28724741275862.

Reference computation (per batch b):
    dec_part[i,o] = dec[b] @ W_dec.T          # [64, 512]
    enc_part[j,o] = enc[b] @ W_enc.T          # [512, 512]
    logits[i,j,o] = dec_part[i,o] + enc_part[j,o] + bias[o]
    alpha = log_softmax(logits, axis=o)
    ctx[i,o] = sum_j alpha[i,j,o] * enc[b][j,o]

Factorization (exact in fp32, ~2e-3 rms with bf16 operands):
    LSE[i,j] = ln sum_o exp(dec_part[i,o]) * exp(enc_part[j,o] + b[o])
    ctx[i,o] = dec_part[i,o]*S[o] + C'[o] - sum_j LSE[i,j]*enc[j,o]
      S[o]  = sum_j enc[j,o]
      C'[o] = sum_j (enc_part[j,o] + b[o]) * enc[j,o]

Pipeline (per core = per batch; everything transposed: features on
partitions, 4 feature blocks of 128):
    A1 (PE):   enc_part^T = W_enc^T.T @ enc^T       ob-major, stops staggered
    ee (ACT):  exp(enc_part^T + b)  per ob as A1 blocks complete
    A2 (PE):   dec_part^T;  ed (ACT): exp(dec_part^T)  single op
    C  (PE):   S_mat^T[j,i] = ee.T @ ed
    Ln (ACT):  LSE^T = ln(S_mat^T)  single op
    S  (Pool): reduce_sum(enc^T) -> se;  C' fused into ONE
               scalar_tensor_tensor per ob (out=(pp+b)*encT, accum=C'),
               split 2/2 across DVE and Pool
    preload (DVE/Pool): pc[ob] = dec_part^T*se + C'  written into PSUM
    E  (PE):   pc[ob] += (-enc).T @ LSE^T   (start=False accumulates onto
               the preload; host negates encN) -> pc holds final ctx^T
    copy+DMA:  pc -> SBUF f32 -> DRAM (p-major flat, 2 rings)

DMA priority: encT + W_enc(ob-major) stream first on the two HWDGE rings;
blkA (W_dec|decT) + encN gated behind them on the gpsimd ring.

Sharding: data-parallel over batch B=8 across the 8 cores; W/b replicated;
no collectives. Self-contained: hardcodes B=8, T_dec=64, T_enc=512, H2=512.
"""

import sys

for _p in ("/opt/trn_rl_repo",):
    if _p not in sys.path:
        sys.path.insert(0, _p)

import numpy as np
import ml_dtypes

import concourse.bass as bass
import concourse.tile as tile
from concourse import bacc, mybir
from concourse.bass_utils import run_bass_kernel_spmd

B, T_DEC, T_ENC, H2 = 8, 64, 512, 512
P = 128  # SBUF partitions
NB = H2 // P  # 4 feature blocks
BA_W = H2 + T_DEC  # 576: blkA row width (W_dec row | decT row)

N_WARMUP = 7  # PE HAM warmup matmuls (512-wide junk)

BF16 = mybir.dt.bfloat16
F32 = mybir.dt.float32
AF = mybir.ActivationFunctionType
ALU = mybir.AluOpType

_CACHE = {}

from contextlib import ExitStack


def build_raw(bacc, mybir, bass):
    BF16 = mybir.dt.bfloat16
    F32 = mybir.dt.float32
    AF = mybir.ActivationFunctionType
    ALU = mybir.AluOpType

    nc = bacc.Bacc(None, target_bir_lowering=False)

    # DRAM inputs, all p-major so every DMA is one long run per partition.
    encTd = nc.dram_tensor("encT", [NB, P, T_ENC], BF16, kind="ExternalInput")
    wobd = nc.dram_tensor("wob", [NB, P, H2], BF16, kind="ExternalInput")
    blkAd = nc.dram_tensor("blkA", [P, NB, BA_W], BF16, kind="ExternalInput")
    encNd = nc.dram_tensor("encN", [P, NB, H2], BF16, kind="ExternalInput")  # NEGATED
    b4d = nc.dram_tensor("b4", [P, NB], F32, kind="ExternalInput")
    outd = nc.dram_tensor("out", [P, NB, T_DEC], F32, kind="ExternalOutput")

    with ExitStack() as ctx:
        ec = ctx.enter_context
        # ---- SBUF ----
        eTt = ec(nc.sbuf_tensor("eTt", [P, NB, T_ENC], BF16))  # enc^T by d-block
        wob = ec(nc.sbuf_tensor("wobs", [P, NB, H2], BF16))  # W_enc^T ob-major
        bt = ec(nc.sbuf_tensor("bt", [P, NB, BA_W], BF16))  # W_dec^T | dec^T
        eN = ec(nc.sbuf_tensor("eN", [P, NB, H2], BF16))  # -enc natural
        b4 = ec(nc.sbuf_tensor("b4s", [P, NB], F32))
        ee = ec(nc.sbuf_tensor("ee", [P, NB, T_ENC], BF16))  # exp(enc_part^T+b)
        ed = ec(nc.sbuf_tensor("ed", [P, NB, T_DEC], BF16))  # exp(dec_part^T)
        lt = ec(nc.sbuf_tensor("lt", [P, NB, T_DEC], BF16))  # LSE^T
        se = ec(nc.sbuf_tensor("se", [P, NB], F32))  # S_enc
        cp = ec(nc.sbuf_tensor("cp", [P, NB], F32))  # C0'
        fx = ec(nc.sbuf_tensor("fx", [P, NB], F32))  # b*S + C0'
        junkD = ec(nc.sbuf_tensor("junkD", [P, NB, T_ENC], BF16))  # stt sinks (DVE)
        ctxo = ec(nc.sbuf_tensor("ctxo", [P, NB, T_DEC], F32))
        jbf = ec(nc.sbuf_tensor("jbf", [P, T_ENC], BF16))  # PE warmup junk
        wj = ec(nc.sbuf_tensor("wj", [P, NB], F32))  # ACT warmup junk
        # ---- PSUM (8 banks exactly) ----
        pp = [ec(nc.psum_tensor(f"pp{o}", [P, T_ENC], F32)) for o in range(NB)]
        pd = ec(nc.psum_tensor("pd", [P, NB, T_DEC], F32))
        ps = ec(nc.psum_tensor("ps", [P, NB, T_DEC], F32))
        pcA = ec(nc.psum_tensor("pcA", [P, 2, T_DEC], F32))
        pcB = ec(nc.psum_tensor("pcB", [P, 2, T_DEC], F32))

        def pc(ob):
            return (pcA if ob < 2 else pcB)[:, ob % 2, :]

        # ---- semaphores ----
        jz = ec(nc.semaphore("jz"))
        dT = ec(nc.semaphore("dT"))  # encT chunks (4 x +16)
        dW = [ec(nc.semaphore(f"dW{o}")) for o in range(NB)]  # wob chunks
        dA = ec(nc.semaphore("dA"))  # blkA
        dN = ec(nc.semaphore("dN"))  # encN
        dB = ec(nc.semaphore("dB"))  # b4
        pe = ec(nc.semaphore("pe"))  # PE stops: A1 1-4, A2 5-8, C 9-12, E 13-16
        ac = ec(nc.semaphore("ac"))  # ACT: warm 1, ee 2-5, ed 6, ln 7
        dv = ec(nc.semaphore("dv"))  # DVE progress counter
        dO = ec(nc.semaphore("dO"))  # out DMAs (2 x +16)

        with nc.Block(no_gpsimd_drain=True) as block:

            @block.sync
            def _(sync):
                # priority stream part 1 on the sync HWDGE ring
                sync.dma_start(out=eTt[:, 0, :], in_=encTd[0, :, :]).then_inc(dT, 16)
                sync.dma_start(out=eTt[:, 2, :], in_=encTd[2, :, :]).then_inc(dT, 16)
                sync.dma_start(out=wob[:, 0, :], in_=wobd[0, :, :]).then_inc(dW[0], 16)
                sync.dma_start(out=wob[:, 2, :], in_=wobd[2, :, :]).then_inc(dW[2], 16)
                # output halves as soon as the copies land
                sync.wait_ge(dv, 20)
                sync.dma_start(out=outd[:, 0:2, :], in_=ctxo[:, 0:2, :]).then_inc(
                    dO, 16
                )
                sync.wait_ge(dv, 22)
                sync.dma_start(out=outd[:, 2:NB, :], in_=ctxo[:, 2:NB, :]).then_inc(
                    dO, 16
                )
                sync.wait_ge(dO, 32)

            @block.scalar
            def _(scalar):
                # priority stream part 2 on the scalar HWDGE ring
                scalar.dma_start(out=b4[:, :], in_=b4d[:, :]).then_inc(dB, 16)
                scalar.dma_start(out=eTt[:, 1, :], in_=encTd[1, :, :]).then_inc(dT, 16)
                scalar.dma_start(out=eTt[:, 3, :], in_=encTd[3, :, :]).then_inc(dT, 16)
                scalar.dma_start(out=wob[:, 1, :], in_=wobd[1, :, :]).then_inc(
                    dW[1], 16
                )
                scalar.dma_start(out=wob[:, 3, :], in_=wobd[3, :, :]).then_inc(
                    dW[3], 16
                )
                # Exp table warmup during the DMA phase.
                scalar.activation(wj[:, 0:1], wj[:, 3:4], AF.Exp, scale=0.0).then_inc(
                    ac, 1
                )  # ac=1
                # ee[ob] = exp(enc_part^T + b) as A1 blocks complete
                scalar.wait_ge(dB, 16)
                for ob in range(NB):
                    scalar.wait_ge(pe, 1 + ob)
                    scalar.activation(
                        ee[:, ob, :],
                        pp[ob][:, :],
                        AF.Exp,
                        bias=b4[:, ob : ob + 1],
                    ).then_inc(ac, 1)  # ac 2..5
                # ed = exp(dec_part^T)
                scalar.wait_ge(pe, 2 * NB)
                for ob in range(NB):
                    mm = scalar.activation(ed[:, ob, :], pd[:, ob, :], AF.Exp)
                    if ob == NB - 1:
                        mm.then_inc(ac, 1)  # ac=6
                # Ln table pull-forward (2nd and last table load)
                scalar.activation(wj[:, 1:2], wj[:, 3:4], AF.Ln, bias=1.0, scale=0.0)
                # LSE^T = ln(S^T)
                scalar.wait_ge(pe, 3 * NB)
                for jb in range(NB):
                    mm = scalar.activation(lt[:, jb, :], ps[:, jb, :], AF.Ln)
                    if jb == NB - 1:
                        mm.then_inc(ac, 1)  # ac=7

            @block.gpsimd
            def _(gpsimd):
                # second-wave DMAs
                gpsimd.dma_start(out=bt[:, :, :], in_=blkAd[:, :, :]).then_inc(dA, 16)
                gpsimd.dma_start(out=eN[:, :, :], in_=encNd[:, :, :]).then_inc(dN, 16)

            @block.tensor
            def _(tensor):
                # HAM warmup on junk data
                tensor.wait_ge(jz, 1)
                for k in range(N_WARMUP):
                    tensor.matmul(
                        pp[k % NB][:, :],
                        lhsT=jbf[:, 0:P],
                        rhs=jbf[:, :],
                        start=True,
                        stop=True,
                    )
                # A1: enc_part^T[ob] += W_enc^T.T @ enc^T, ob-major
                tensor.wait_ge(dT, 64)
                for ob in range(NB):
                    tensor.wait_ge(dW[ob], 16)
                    for db in range(NB):
                        mm = tensor.matmul(
                            pp[ob][:, :],
                            lhsT=wob[:, ob, db * P : (db + 1) * P],
                            rhs=eTt[:, db, :],
                            start=(db == 0),
                            stop=(db == NB - 1),
                        )
                        if db == NB - 1:
                            mm.then_inc(pe, 1)  # pe 1..4
                # A2: dec_part^T
                tensor.wait_ge(dA, 16)
                for ob in range(NB):
                    for db in range(NB):
                        mm = tensor.matmul(
                            pd[:, ob, :],
                            lhsT=bt[:, db, ob * P : (ob + 1) * P],
                            rhs=bt[:, db, H2 : H2 + T_DEC],
                            start=(db == 0),
                            stop=(db == NB - 1),
                        )
                        if db == NB - 1:
                            mm.then_inc(pe, 1)  # pe 5..8
                # C: S^T[jb] += ee[ob].T @ ed[ob]
                tensor.wait_ge(ac, 6)  # all ee + ed
                for jb in range(NB):
                    for ob in range(NB):
                        mm = tensor.matmul(
                            ps[:, jb, :],
                            lhsT=ee[:, ob, jb * P : (jb + 1) * P],
                            rhs=ed[:, ob, :],
                            start=(ob == 0),
                            stop=(ob == NB - 1),
                        )
                        if ob == NB - 1:
                            mm.then_inc(pe, 1)  # pe 9..12
                # E: pc[ob] += (-enc).T @ LSE^T onto the preloaded combine
                tensor.wait_ge(dN, 16)
                tensor.wait_ge(ac, 7)  # ln done
                for ob in range(NB):
                    for jb in range(NB):
                        mm = tensor.matmul(
                            pc(ob),
                            lhsT=eN[:, jb, ob * P : (ob + 1) * P],
                            rhs=lt[:, jb, :],
                            start=(jb == 0),
                            stop=(jb == NB - 1),
                        )
                        if jb == NB - 1:
                            mm.then_inc(pe, 1)  # pe 13..16

            @block.vector
            def _(vector):
                vector.memset(jbf[:, :], 0.0).then_inc(jz, 1)
                # S_enc = row-sum of enc^T
                vector.wait_ge(dT, 64)
                for ob in range(NB):
                    vector.reduce_sum(
                        out=se[:, ob : ob + 1],
                        in_=eTt[:, ob, :],
                        axis=mybir.AxisListType.X,
                    ).then_inc(dv, 1)  # dv 1..4
                # C0' = sum_j enc_part^T * enc^T, pipelined behind the exps
                for ob in range(NB):
                    vector.wait_ge(ac, 2 + ob)  # after ee[ob]
                    vector.tensor_tensor(
                        out=junkD[:, ob, :],
                        in0=pp[ob][:, :],
                        in1=eTt[:, ob, :],
                        op=ALU.mult,
                    ).then_inc(dv, 1)  # dv 5,7,9,11
                    vector.wait_ge(dv, 5 + 2 * ob)
                    vector.reduce_sum(
                        out=cp[:, ob : ob + 1],
                        in_=junkD[:, ob, :],
                        axis=mybir.AxisListType.X,
                    ).then_inc(dv, 1)  # dv 6,8,10,12
                # fix = b*S + C0'   (one [P,NB] two-op pass)
                vector.wait_ge(dB, 16)
                vector.wait_ge(dv, 12)
                vector.tensor_tensor(
                    out=fx[:, :], in0=se[:, :], in1=b4[:, :], op=ALU.mult
                ).then_inc(dv, 1)  # dv 13
                vector.wait_ge(dv, 13)
                vector.tensor_tensor(
                    out=fx[:, :], in0=fx[:, :], in1=cp[:, :], op=ALU.add
                ).then_inc(dv, 1)  # dv 14
                # ctmp = dec_part^T*se + fix  (into SBUF)
                vector.wait_ge(dv, 14)
                vector.wait_ge(pe, 2 * NB)  # dec_part^T ready
                for ob in range(NB):
                    vector.tensor_scalar(
                        out=ctxo[:, ob, :],
                        in0=pd[:, ob, :],
                        scalar1=se[:, ob : ob + 1],
                        scalar2=fx[:, ob : ob + 1],
                        op0=ALU.mult,
                        op1=ALU.add,
                    ).then_inc(dv, 1)  # dv 15..18
                # ctx = ctmp + (-ctx2)
                vector.wait_ge(pe, 14)  # pcA fully accumulated
                for ob in range(2):
                    vector.wait_ge(dv, 15 + ob)
                    vector.tensor_tensor(
                        out=ctxo[:, ob, :],
                        in0=ctxo[:, ob, :],
                        in1=pcA[:, ob, :],
                        op=ALU.add,
                    ).then_inc(dv, 1)  # dv 19,20
                vector.wait_ge(pe, 16)  # pcB fully accumulated
                for ob in range(2, NB):
                    vector.wait_ge(dv, 15 + ob)
                    vector.tensor_tensor(
                        out=ctxo[:, ob, :],
                        in0=ctxo[:, ob, :],
                        in1=pcB[:, ob % 2, :],
                        op=ALU.add,
                    ).then_inc(dv, 1)  # dv 21,22

        nc.finalize()
    return nc


def _build_nc():
    return build_raw(bacc, mybir, bass)


def _prep_in_maps(encoderOutput, decoderInput, W, b):
    bf = ml_dtypes.bfloat16
    WT = np.ascontiguousarray(np.asarray(W, np.float32).T)  # [2H, H2]
    WdecT = WT[:H2]  # [H2(d), H2(o)]
    WencT = WT[H2:]  # [H2(d), H2(o)]
    # wob[ob, p, db*128+c] = WencT[db*128+p, ob*128+c]
    wob = np.ascontiguousarray(
        WencT.reshape(NB, P, NB, P).transpose(2, 1, 0, 3).reshape(NB, P, H2)
    ).astype(bf)
    b4 = np.ascontiguousarray(np.asarray(b, np.float32).reshape(NB, P).T)
    WdecT_r = WdecT.reshape(NB, P, H2)
    in_maps = []
    for core in range(B):
        e = np.asarray(encoderOutput[core], np.float32)
        d = np.asarray(decoderInput[core], np.float32)
        encT = np.ascontiguousarray(e.T).reshape(NB, P, T_ENC)
        # blkA[p, db, :] = [WdecT row | decT row]
        bA = np.empty((P, NB, BA_W), np.float32)
        bA[:, :, :H2] = WdecT_r.transpose(1, 0, 2)
        bA[:, :, H2:] = d.T.reshape(NB, P, T_DEC).transpose(1, 0, 2)
        # encN[p, jb, o] = -enc[jb*128+p, o]   (negated for the E accumulate)
        eNn = -e.reshape(NB, P, H2).transpose(1, 0, 2)
        in_maps.append(
            {
                "encT": encT.astype(bf),
                "wob": wob,
                "blkA": bA.astype(bf),
                "encN": eNn.astype(bf),
                "b4": b4,
            }
        )
    return in_maps


def _unshard_single(arr):
    # out[p, ob, i] = ctx^T[ob*128+p, i]  ->  ctx [T_dec, H2]
    a = np.asarray(arr, np.float32).reshape(P, NB, T_DEC)
    return a.transpose(1, 0, 2).reshape(H2, T_DEC).T


def kernel(encoderOutput, decoderInput, W, b, _trace=False):
    if "nc" not in _CACHE:
        _CACHE["nc"] = _build_nc()
    nc = _CACHE["nc"]
    in_maps = _prep_in_maps(encoderOutput, decoderInput, W, b)
    res = run_bass_kernel_spmd(nc, in_maps, core_ids=list(range(B)), trace=_trace)
    outs = np.stack([_unshard_single(r["out"]) for r in res.results])
    if _trace:
        _CACHE["last_result"] = res
    return outs
